# revision 4
# baseline (speedup 1.0000x reference)
"""LIF bank kernel for 8 trn2 NeuronCores.

Data-parallel over batch B=32 -> 4 samples/core. Host transposes h -> hT (C,T)
and gain-folds W into W'^T (C,K) + bias2 (free). Device: fp32 PE matmul
produces I^T[k,t] per sample in PSUM; ACT evacuates with bias-add into a
t-major interleaved SBUF layout I_mega[p, 16*t + kt*4 + b]; then 1024 fused
per-step DVE instructions (custom Spec op: V' = u - (u>=1), u = alpha*V + I)
run the LIF scan with the full per-core state [128, 16] per step. V streams
out raw; the host derives S = (u >= 1) bitwise-identically from V and I
(same fp32 elementwise ops) and deinterleaves all outputs.
"""

import numpy as np

import concourse.bass as bass
import concourse.bacc as bacc
import concourse.mybir as mybir
from concourse.bass_utils import run_bass_kernel_spmd
from concourse.tile import TileContext

from lif_op import LIF_STEP_ANT, register_step_op

register_step_op()

ALPHA = 0.95
B, T, C, K = 32, 1024, 512, 512
NCORES = 8
BL = B // NCORES  # 4
NKT = K // 128
NCT = C // 128
TC = 512
NS = BL * NKT  # 16 series per partition
NI = T * NS  # I_mega free size
PAD = NS  # V zero-prefix columns

_NC_CACHE = {}


def build():
    if "nc" in _NC_CACHE:
        return _NC_CACHE["nc"]
    f32 = mybir.dt.float32
    nc = bacc.Bacc("TRN2", target_bir_lowering=False, debug=False, num_devices=NCORES)
    hT = nc.dram_tensor("hT", [BL, C, T], f32, kind="ExternalInput")
    wt = nc.dram_tensor("wt", [C, K], f32, kind="ExternalInput")
    bias2 = nc.dram_tensor("bias2", [128, NKT], f32, kind="ExternalInput")
    I_out = nc.dram_tensor("I_out", [128, NI], f32, kind="ExternalOutput")
    V_out = nc.dram_tensor("V_out", [128, PAD + NI], f32, kind="ExternalOutput")

    with TileContext(nc) as tc:
        with (
            tc.tile_pool(name="wpool", bufs=1) as wpool,
            tc.tile_pool(name="hpool", bufs=2) as hpool,
            tc.tile_pool(name="mega", bufs=1) as mega,
            tc.tile_pool(name="psum", bufs=4, space="PSUM") as psum_pool,
        ):
            bias_t = wpool.tile([128, NKT], f32, tag="bias")
            nc.sync.dma_start(bias_t[:, :], bias2[:, :])
            wtiles = []
            for ct in range(NCT):
                wtile = wpool.tile([128, K], f32, tag=f"w{ct}")
                nc.sync.dma_start(wtile[:, :], wt[ct * 128 : (ct + 1) * 128, :])
                wtiles.append(wtile)

            imega = mega.tile([128, NI], f32, tag="imega")
            vmega = mega.tile([128, PAD + NI], f32, tag="vmega")
            nc.vector.memset(vmega[:, 0:PAD], 0.0)

            iap = imega[:, :]
            vap = vmega[:, :]
            pstep = iap.ap[0][0]
            vstep = vap.ap[0][0]

            for tci in range(T // TC):
                for b in range(BL):
                    htiles = []
                    for ct in range(NCT):
                        ht = hpool.tile([128, TC], f32, tag=f"h{ct}")
                        nc.sync.dma_start(
                            ht[:, :],
                            hT[b, ct * 128 : (ct + 1) * 128, tci * TC : (tci + 1) * TC],
                        )
                        htiles.append(ht)
                    for kt in range(NKT):
                        ps = psum_pool.tile([128, TC], f32, tag="ps")
                        for ct in range(NCT):
                            nc.tensor.matmul(
                                ps[:, :],
                                wtiles[ct][:, kt * 128 : (kt + 1) * 128],
                                htiles[ct][:, :],
                                start=(ct == 0),
                                stop=(ct == NCT - 1),
                            )
                        # strided dst: cols (tci*TC + t')*NS + kt*BL + b
                        dst = bass.AP(
                            iap.tensor,
                            iap.offset + tci * TC * NS + kt * BL + b,
                            [[pstep, 128], [NS, TC]],
                        )
                        nc.scalar.activation(
                            dst,
                            ps[:, :],
                            mybir.ActivationFunctionType.Identity,
                            bias=bias_t[:, kt : kt + 1],
                        )
                # scan steps for this tci chunk
                for t in range(tci * TC, (tci + 1) * TC):
                    nc.vector._custom_dve(
                        LIF_STEP_ANT,
                        out=bass.AP(
                            vap.tensor,
                            vap.offset + PAD + t * NS,
                            [[vstep, 128], [1, NS]],
                        ),
                        in0=bass.AP(
                            vap.tensor, vap.offset + t * NS, [[vstep, 128], [1, NS]]
                        ),
                        in1=bass.AP(
                            iap.tensor, iap.offset + t * NS, [[pstep, 128], [1, NS]]
                        ),
                        s0=ALPHA,
                    )
                nc.sync.dma_start(
                    I_out[:, tci * TC * NS : (tci + 1) * TC * NS],
                    imega[:, tci * TC * NS : (tci + 1) * TC * NS],
                )
                nc.sync.dma_start(
                    V_out[:, tci * TC * NS : PAD + (tci + 1) * TC * NS - PAD],
                    vmega[:, tci * TC * NS : (tci + 1) * TC * NS],
                )
            nc.sync.dma_start(V_out[:, NI : NI + PAD], vmega[:, NI : NI + PAD])
    nc.compile()
    _NC_CACHE["nc"] = nc
    return nc


def kernel(h, W, b_lin, gain, bias, _want_results=None):
    h = np.asarray(h, np.float32)
    W = np.asarray(W, np.float32)
    b_lin = np.asarray(b_lin, np.float32)
    gain = np.asarray(gain, np.float32)
    bias = np.asarray(bias, np.float32)

    Wp = (W * gain[:, None]).T  # (C, K)
    bias2 = (b_lin * gain + bias).reshape(NKT, 128).T  # (128, NKT)
    wt_np = np.ascontiguousarray(Wp, dtype=np.float32)
    bias2_np = np.ascontiguousarray(bias2, dtype=np.float32)

    in_maps = []
    for c in range(NCORES):
        hc = h[c * BL : (c + 1) * BL]
        hTc = np.ascontiguousarray(hc.transpose(0, 2, 1))
        in_maps.append({"hT": hTc, "wt": wt_np, "bias2": bias2_np})

    nc = build()
    res = run_bass_kernel_spmd(nc, in_maps, list(range(NCORES)))
    if _want_results is not None:
        _want_results.append(res)

    S = np.empty((B, T, K), np.float32)
    Vt = np.empty((B, T, K), np.float32)
    I = np.empty((B, T, K), np.float32)
    for c in range(NCORES):
        r = res.results[c]
        sl = slice(c * BL, (c + 1) * BL)
        # raw[p, t*16 + kt*4 + b] -> [b, t, kt*128 + p]
        iraw = r["I_out"].reshape(128, T, NKT, BL)
        vraw = r["V_out"][:, PAD:].reshape(128, T, NKT, BL)
        I[sl] = iraw.transpose(3, 1, 2, 0).reshape(BL, T, K)
        Vt[sl] = vraw.transpose(3, 1, 2, 0).reshape(BL, T, K)
    # S derived bitwise-identically: u = fl(fl(alpha*V_prev) + I); s = u >= 1
    Vprev = np.concatenate([np.zeros((B, 1, K), np.float32), Vt[:, :-1]], axis=1)
    u = (np.float32(ALPHA) * Vprev) + I
    S[:] = (u >= np.float32(1.0)).astype(np.float32)
    return S, Vt, I


# revision 5
# speedup vs baseline: 1.1361x; 1.1361x over previous
"""LIF bank kernel for 8 trn2 NeuronCores.

Data-parallel over batch B=32 -> 4 samples/core. Host transposes h -> hT (C,T)
and gain-folds W into W'^T (C,K) + bias2 (free). Device: fp32 PE matmul
produces I^T[k,t] per sample in PSUM; ACT evacuates with bias-add into a
t-major interleaved SBUF layout I_mega[p, 16*t + kt*4 + b]; then 1024 fused
per-step DVE instructions (custom Spec op: V' = u - (u>=1), u = alpha*V + I)
run the LIF scan with the full per-core state [128, 16] per step. V streams
out raw; the host derives S = (u >= 1) bitwise-identically from V and I
(same fp32 elementwise ops) and deinterleaves all outputs.
"""

import numpy as np
from dataclasses import dataclass

import concourse.bass as bass
import concourse.bacc as bacc
import concourse.mybir as mybir
from concourse.bass_utils import run_bass_kernel_spmd
from concourse.tile import TileContext
from concourse import dve_ops
from concourse.dve_ops import DveOp
from concourse.dve_spec import Spec, Src0, Src1, C0, One, lower as _lower
from concourse.dve_uop import DveOpSpec


@dataclass(frozen=True)
class _LegalDveOp(DveOp):
    """DveOp compiled via production lower(), without a pinned sha."""

    def compile(self, ver):
        key = (self.name, ver)
        cache = dve_ops._COMPILE_CACHE
        if (r := cache.get(key)) is not None:
            return r
        result = DveOpSpec(
            name=self.name,
            opcode=dve_ops.get_dve_sub_opcode(self.name),
            uops=_lower(self.spec, ver=ver),
            rd1_en=True,
        )
        cache[key] = result
        return result


def _step_ref(in0, in1, s0, s1, imm2):
    a = s0 if not isinstance(s0, np.ndarray) else s0.reshape(-1, 1)
    u = (in0.astype(np.float32) * np.float32(a)) + in1.astype(np.float32)
    return u - (u >= np.float32(1.0)).astype(np.float32)


def _mk_step():
    u_expr = Src0 * C0 + Src1
    return _LegalDveOp(
        name="LIF_STEP_ANT",
        spec=Spec(body=u_expr - (u_expr >= One), reference=_step_ref),
        subdim=False,
        uops_sha={},
    )


LIF_STEP_ANT = _mk_step()


def register_step_op():
    op = LIF_STEP_ANT
    if op.name in dve_ops._SUB_OPCODE_FOR_NAME:
        return
    row = dve_ops._CUSTOM_DVE_ROW_BASE + len(dve_ops.OPS)
    assert row < 0x20
    dve_ops.OPS.append(op)
    dve_ops._SUB_OPCODE_FOR_NAME[op.name] = row
    dve_ops.CUSTOM_DVE_SPECS[op.name] = op.spec

register_step_op()

ALPHA = 0.95
B, T, C, K = 32, 1024, 512, 512
NCORES = 8
BL = B // NCORES  # 4
NKT = K // 128
NCT = C // 128
TC = 512
NS = BL * NKT  # 16 series per partition
NI = T * NS  # I_mega free size
PAD = NS  # V zero-prefix columns

_NC_CACHE = {}


def build():
    if "nc" in _NC_CACHE:
        return _NC_CACHE["nc"]
    f32 = mybir.dt.float32
    nc = bacc.Bacc("TRN2", target_bir_lowering=False, debug=False, num_devices=NCORES)
    hT = nc.dram_tensor("hT", [BL, C, T], f32, kind="ExternalInput")
    wt = nc.dram_tensor("wt", [C, K], f32, kind="ExternalInput")
    bias2 = nc.dram_tensor("bias2", [128, NKT], f32, kind="ExternalInput")
    I_out = nc.dram_tensor("I_out", [128, NI], f32, kind="ExternalOutput")
    V_out = nc.dram_tensor("V_out", [128, PAD + NI], f32, kind="ExternalOutput")

    with TileContext(nc) as tc:
        with (
            tc.tile_pool(name="wpool", bufs=1) as wpool,
            tc.tile_pool(name="hpool", bufs=2) as hpool,
            tc.tile_pool(name="mega", bufs=1) as mega,
            tc.tile_pool(name="psum", bufs=4, space="PSUM") as psum_pool,
        ):
            bias_t = wpool.tile([128, NKT], f32, tag="bias")
            nc.sync.dma_start(bias_t[:, :], bias2[:, :])
            wtiles = []
            for ct in range(NCT):
                wtile = wpool.tile([128, K], f32, tag=f"w{ct}")
                nc.sync.dma_start(wtile[:, :], wt[ct * 128 : (ct + 1) * 128, :])
                wtiles.append(wtile)

            imega = mega.tile([128, NI], f32, tag="imega")
            vmega = mega.tile([128, PAD + NI], f32, tag="vmega")
            nc.vector.memset(vmega[:, 0:PAD], 0.0)

            iap = imega[:, :]
            vap = vmega[:, :]
            pstep = iap.ap[0][0]
            vstep = vap.ap[0][0]

            for tci in range(T // TC):
                for b in range(BL):
                    htiles = []
                    for ct in range(NCT):
                        ht = hpool.tile([128, TC], f32, tag=f"h{ct}")
                        nc.sync.dma_start(
                            ht[:, :],
                            hT[b, ct * 128 : (ct + 1) * 128, tci * TC : (tci + 1) * TC],
                        )
                        htiles.append(ht)
                    for kt in range(NKT):
                        ps = psum_pool.tile([128, TC], f32, tag="ps")
                        for ct in range(NCT):
                            nc.tensor.matmul(
                                ps[:, :],
                                wtiles[ct][:, kt * 128 : (kt + 1) * 128],
                                htiles[ct][:, :],
                                start=(ct == 0),
                                stop=(ct == NCT - 1),
                            )
                        # strided dst: cols (tci*TC + t')*NS + kt*BL + b
                        dst = bass.AP(
                            iap.tensor,
                            iap.offset + tci * TC * NS + kt * BL + b,
                            [[pstep, 128], [NS, TC]],
                        )
                        nc.scalar.activation(
                            dst,
                            ps[:, :],
                            mybir.ActivationFunctionType.Identity,
                            bias=bias_t[:, kt : kt + 1],
                        )
                # scan steps for this tci chunk
                for t in range(tci * TC, (tci + 1) * TC):
                    nc.vector._custom_dve(
                        LIF_STEP_ANT,
                        out=bass.AP(
                            vap.tensor,
                            vap.offset + PAD + t * NS,
                            [[vstep, 128], [1, NS]],
                        ),
                        in0=bass.AP(
                            vap.tensor, vap.offset + t * NS, [[vstep, 128], [1, NS]]
                        ),
                        in1=bass.AP(
                            iap.tensor, iap.offset + t * NS, [[pstep, 128], [1, NS]]
                        ),
                        s0=ALPHA,
                    )
                nc.sync.dma_start(
                    I_out[:, tci * TC * NS : (tci + 1) * TC * NS],
                    imega[:, tci * TC * NS : (tci + 1) * TC * NS],
                )
                nc.sync.dma_start(
                    V_out[:, tci * TC * NS : PAD + (tci + 1) * TC * NS - PAD],
                    vmega[:, tci * TC * NS : (tci + 1) * TC * NS],
                )
            nc.sync.dma_start(V_out[:, NI : NI + PAD], vmega[:, NI : NI + PAD])
    nc.compile()
    _NC_CACHE["nc"] = nc
    return nc


def kernel(h, W, b_lin, gain, bias, _want_results=None):
    h = np.asarray(h, np.float32)
    W = np.asarray(W, np.float32)
    b_lin = np.asarray(b_lin, np.float32)
    gain = np.asarray(gain, np.float32)
    bias = np.asarray(bias, np.float32)

    Wp = (W * gain[:, None]).T  # (C, K)
    bias2 = (b_lin * gain + bias).reshape(NKT, 128).T  # (128, NKT)
    wt_np = np.ascontiguousarray(Wp, dtype=np.float32)
    bias2_np = np.ascontiguousarray(bias2, dtype=np.float32)

    in_maps = []
    for c in range(NCORES):
        hc = h[c * BL : (c + 1) * BL]
        hTc = np.ascontiguousarray(hc.transpose(0, 2, 1))
        in_maps.append({"hT": hTc, "wt": wt_np, "bias2": bias2_np})

    nc = build()
    res = run_bass_kernel_spmd(nc, in_maps, list(range(NCORES)))
    if _want_results is not None:
        _want_results.append(res)

    S = np.empty((B, T, K), np.float32)
    Vt = np.empty((B, T, K), np.float32)
    I = np.empty((B, T, K), np.float32)
    for c in range(NCORES):
        r = res.results[c]
        sl = slice(c * BL, (c + 1) * BL)
        # raw[p, t*16 + kt*4 + b] -> [b, t, kt*128 + p]
        iraw = r["I_out"].reshape(128, T, NKT, BL)
        vraw = r["V_out"][:, PAD:].reshape(128, T, NKT, BL)
        I[sl] = iraw.transpose(3, 1, 2, 0).reshape(BL, T, K)
        Vt[sl] = vraw.transpose(3, 1, 2, 0).reshape(BL, T, K)
    # S derived bitwise-identically: u = fl(fl(alpha*V_prev) + I); s = u >= 1
    Vprev = np.concatenate([np.zeros((B, 1, K), np.float32), Vt[:, :-1]], axis=1)
    u = (np.float32(ALPHA) * Vprev) + I
    S[:] = (u >= np.float32(1.0)).astype(np.float32)
    return S, Vt, I
